# revision 41
# baseline (speedup 1.0000x reference)
"""Trainium2 Bass kernel for AFT-style sparse attention (nn_AFTKVR).

Reference computation (per batch b):
    q,k,v = x @ W{q,k,v}.T          # [T=1024, D=256], H=4 heads x d=64
    ew = exp(wbias)                  # [T, K=63] neighbor weights
    ek = exp(k); kv = ek * v
    num[t] = sum_k ew[t,k] * kv[idx[t,k]]   (idx = row+col neighbors on 32x32 grid)
    den[t] = sum_k ew[t,k] * ek[idx[t,k]]
    out = sigmoid(q) * num / den

Sharding: 8 cores = 4 batches x 2 head-pairs (128 features each). No collectives.

Per-core device algorithm (all matmul operands bf16, fp32 PSUM accumulation):
  - each dma_start's trigger costs ~700ns on the ISSUING engine's queue,
    and the SDMA engines drain one transfer's packet set before switching
    queues, so arrival order follows descriptor-ready order.  Assignment:
      scalar: wb1 [wq|wkv] (192KB), xt2 token-half 0 (256KB)
      sync:   xt half 0, wrow_c, wcol_c (compact neighbor weights, 64KB
              each), xt half 1, xt2 half 1
    xt2 (x in grid-transposed token order t' = c*32+r) is shipped
    pre-permuted from the host: the DMA stream has slack while every
    on-chip engine does not.
  - the block-diagonal neighbor-weight matrices are expanded ON-DEVICE
    from the compact [128,256] form (Pool memsets + 8 strided DVE copies
    in DVE's idle early window), saving 384KB of HBM streaming per core.
  - PE warm-up: dummy matmuls keep the PE continuously busy from engine
    start until the first real matmul so the HAM activity monitor sees a
    full busy window as early as possible and releases the clock gate
    (1.2 -> 2.4 GHz) during the projection phase, not after it.
  - q projected feature-major -> qT PSUM; ACT computes eq = exp(-qT).
  - k|v projected token-major per 128-token group (lhsT = xt slice) ->
    ek, kv; same from xt2 -> ekp, kvp (grid-col-major blocks).
  - The 63-neighbor gather+reduce decomposes into block-diagonal matmuls
    accumulated into zeroed PSUM (start=False + skip_group_check):
      row part: denT/numT[f, tok-slice] += {ek,kv}_g.T @ wrow_g
      col part: denT/numT[f, strided]   += {ekp,kvp}_g.T @ wcol_g
    den parts first so the combine's recip chain overlaps the num parts;
    the kvp3 mul (and the two num-col matmuls it feeds) are deferred past
    the first stt+recip pair so the den->recip chain isn't queued behind
    them on DVE.
  - combine (DVE): m1 = (eq+1)*den [scalar_tensor_tensor], rden =
    recip_approx(m1), then out = num*rden (== sigmoid(q)*num/den),
    written feature-major bf16 in quarters, each feeding its own output
    DMA (sync / scalar queues alternating).  Host casts/transposes
    during unshard.
"""

import os
from contextlib import ExitStack

import ml_dtypes
import numpy as np

import concourse.bass as bass
import concourse.tile as tile
from concourse import bacc, mybir
from concourse.bass_utils import run_bass_kernel_spmd

BF = mybir.dt.bfloat16
F32 = mybir.dt.float32
AF = mybir.ActivationFunctionType
ALU = mybir.AluOpType

N = 32          # grid side
T = N * N       # tokens
D = 256         # model dim
F = 128         # features per core (2 heads x 64)
NEG = -1e30     # exp(NEG) == 0

LAST_RESULT = None  # BassKernelResults of the most recent run (for profiling)
_CACHED_NC = None

N_WARM = 14     # dummy matmuls bridging engine start -> first real matmul;
                # sized so the PE never idles >~0.5us before the first real
                # matmul even when the input DMA lands late -- a longer gap
                # resets the HAM activity window and the whole projection
                # phase then runs at 1.2 GHz instead of 2.4


def _build_nc():
    nc = bacc.Bacc("TRN2", target_bir_lowering=False, debug=False)

    xt_ds = [nc.declare_dram_parameter(f"xth{i}", [128, 1024], BF,
                                       isOutput=False) for i in range(2)]
    xt2_ds = [nc.declare_dram_parameter(f"xt2h{i}", [128, 1024], BF,
                                        isOutput=False) for i in range(2)]
    wb1_d = nc.declare_dram_parameter("wb1", [128, 768], BF, isOutput=False)
    wrc_d = nc.declare_dram_parameter("wrc", [128, 256], BF, isOutput=False)
    wcc_d = nc.declare_dram_parameter("wcc", [128, 256], BF, isOutput=False)
    out_d = nc.declare_dram_parameter("out", [128, 1024], BF, isOutput=True)

    from concourse.tile_rust import add_dep_helper

    with tile.TileContext(nc) as tc, ExitStack() as ctx:
        sb = ctx.enter_context(tc.tile_pool(name="sb", bufs=1))
        ps_q = ctx.enter_context(tc.tile_pool(name="ps_q", bufs=1, space="PSUM"))
        ps_kv = ctx.enter_context(tc.tile_pool(name="ps_kv", bufs=3, space="PSUM"))
        ps_g = ctx.enter_context(tc.tile_pool(name="ps_g", bufs=1, space="PSUM"))

        xt = sb.tile([128, 2048], BF, tag="xt")
        xt2 = sb.tile([128, 2048], BF, tag="xt2")
        wb1 = sb.tile([128, 768], BF, tag="wb1")
        wrc = sb.tile([128, 256], BF, tag="wrc")
        wcc = sb.tile([128, 256], BF, tag="wcc")
        wrow_e = sb.tile([128, 1024], BF, tag="wrow_e")
        wcol_e = sb.tile([128, 1024], BF, tag="wcol_e")
        warm = sb.tile([128, 512], BF, tag="warm")
        pre = sb.tile([128, 16], F32, tag="pre")
        ek = sb.tile([128, 1024], BF, tag="ek")
        kv = sb.tile([128, 1024], BF, tag="kv")
        ekp = sb.tile([128, 1024], BF, tag="ekp")
        kvp = sb.tile([128, 1024], BF, tag="kvp")
        eq = sb.tile([128, 1024], F32, tag="eq")
        m1 = sb.tile([128, 1024], F32, tag="m1")
        lnt = sb.tile([128, 1024], F32, tag="lnt")
        rden = sb.tile([128, 1024], F32, tag="rden")
        w2 = sb.tile([128, 1024], BF, tag="w2")

        wq = wb1[:, 0:256]
        wkv = wb1[:, 256:768]

        # input loads: each dma_start's trigger costs ~700ns on the ISSUING
        # engine's queue, and the SDMA engines drain one transfer's packet
        # before switching queues -- so arrival order across queues follows
        # descriptor-ready order, not queue parallelism.  The busy ACT
        # engine gets only wb1; idle sync takes the rest in need-order; the
        # idle Pool engine ships xt2h0 via SWDGE.  xt2 (the grid-transposed
        # copy of x) is shipped pre-permuted from the host.
        nc.scalar.dma_start(out=wb1[:], in_=wb1_d[:])
        nc.scalar.dma_start(out=xt2[:, 0:1024], in_=xt2_ds[0][:])
        nc.sync.dma_start(out=xt[:, 0:1024], in_=xt_ds[0][:])
        nc.sync.dma_start(out=wrc[:], in_=wrc_d[:])
        nc.sync.dma_start(out=wcc[:], in_=wcc_d[:])
        nc.sync.dma_start(out=xt[:, 1024:2048], in_=xt_ds[1][:])
        nc.sync.dma_start(out=xt2[:, 1024:2048], in_=xt2_ds[1][:])

        # PE warm-up: dummy matmuls while the input DMAs stream in, so the
        # HAM clock gate sees a full busy window and releases (1.2 -> 2.4
        # GHz) before (or early in) the projection phase.  The warm tile is
        # zeroed on DVE (first op) so the Pool queue's SWDGE trigger does
        # not delay the PE start.
        nc.vector.memset(warm[:], 0.0)
        for i in range(N_WARM):
            wps = ps_kv.tile([128, 512], F32, tag="kvps")
            nc.tensor.matmul(wps[:, 0:256], warm[:, 0:128], warm[:, 0:256],
                             start=True, stop=True)

        # ACT table preload: a dummy Exp so the exp table load happens
        # while inputs stream in
        nc.scalar.activation(pre[:], warm[:, 0:16], AF.Exp)

        # zero the expanded neighbor-weight tiles on Pool early (off-block
        # entries stay zero) and the grid PSUM accumulators on DVE (its
        # early window); every grid matmul then accumulates with
        # start=False + skip_group_check.
        nc.gpsimd.memset(wrow_e[:], 0.0)
        nc.gpsimd.memset(wcol_e[:], 0.0)
        numT = ps_g.tile([128, 1024], F32, tag="numT")
        denT = ps_g.tile([128, 1024], F32, tag="denT")
        for bank in range(2):
            nc.vector.memset(denT[:, bank * 512:(bank + 1) * 512], 0.0)
        for bank in range(2):
            nc.vector.memset(numT[:, bank * 512:(bank + 1) * 512], 0.0)

        # on-device block-diagonal expansion of the compact neighbor
        # weights: wX_e[rb*32+j, g*128 + rb*32 + c] = wXc[rb*32+j, g*32+c]
        # (8 strided DVE copies in DVE's idle early window)
        wrow_ev = wrow_e[:].rearrange("p (g c) -> p g c", g=8)
        wcol_ev = wcol_e[:].rearrange("p (g c) -> p g c", g=8)
        wrc_v = wrc[:].rearrange("p (g c) -> p g c", g=8)
        wcc_v = wcc[:].rearrange("p (g c) -> p g c", g=8)

        ek_vw = ek[:].rearrange("p (g f) -> p g f", f=128)
        kv_vw = kv[:].rearrange("p (g f) -> p g f", f=128)
        ekp_vw = ekp[:].rearrange("p (g f) -> p g f", f=128)
        kvp_vw = kvp[:].rearrange("p (g f) -> p g f", f=128)

        qp = {}

        def q_proj(nh):
            qp[nh] = ps_q.tile([128, 512], F32, name=f"qp{nh}", tag="qp")
            for kh in range(2):
                nc.tensor.matmul(
                    qp[nh][:],
                    wq[:, kh * 128:(kh + 1) * 128],
                    xt[:, nh * 1024 + kh * 512: nh * 1024 + (kh + 1) * 512],
                    start=(kh == 0), stop=(kh == 1),
                )

        def kv_proj(pr, src, ek_t, kv_t, do_mul=True):
            kvps = ps_kv.tile([128, 512], F32, tag="kvps")
            mm = {}
            for g2 in range(2):
                g = 2 * pr + g2
                for kh in range(2):
                    if src is xt:
                        base = (g // 4) * 1024 + kh * 512 + (g % 4) * 128
                    else:
                        base = kh * 1024 + g * 128
                    lhsT = src[:, base: base + 128]
                    mm[g2, kh] = nc.tensor.matmul(
                        kvps[:, g2 * 256:(g2 + 1) * 256],
                        lhsT,
                        wkv[:, kh * 256:(kh + 1) * 256],
                        start=(g2 == 0 and kh == 0),
                        stop=(g2 == 1 and kh == 1),
                    )
            # keep PSUM zero-region state machine ordering legal: the
            # start=True matmul first, the stop=True matmul last
            add_dep_helper(mm[1, 0].ins, mm[0, 0].ins, reason="psum start first")
            add_dep_helper(mm[1, 1].ins, mm[0, 1].ins, reason="psum stop last")
            kvps_v = kvps[:].rearrange("p (g c) -> p g c", g=2)
            ps_ = slice(2 * pr, 2 * pr + 2)
            nc.scalar.activation(ek_t[:, ps_, :], kvps_v[:, :, 0:128], AF.Exp)
            if do_mul:
                nc.vector.tensor_mul(kv_t[:, ps_, :], ek_t[:, ps_, :],
                                     kvps_v[:, :, 128:256])
            return kvps_v

        # PE order: q0 -> kvA pr0-3 -> q1 -> kvB pr0-3 -> grid (den, num).
        # The weight expansion copies fill DVE's idle window before the
        # kv muls.
        for rb in range(4):
            ps_, cs_ = slice(rb * 32, (rb + 1) * 32), slice(rb * 32, rb * 32 + 32)
            nc.vector.tensor_copy(wrow_ev[ps_, :, cs_], wrc_v[ps_, :, :])
        for rb in range(4):
            ps_, cs_ = slice(rb * 32, (rb + 1) * 32), slice(rb * 32, rb * 32 + 32)
            nc.vector.tensor_copy(wcol_ev[ps_, :, cs_], wcc_v[ps_, :, :])
        q_proj(0)
        kv_proj(0, xt, ek_vw, kv_vw)
        nc.scalar.activation(eq[:, 0:512], qp[0][:], AF.Exp, scale=-1.0)
        kv_proj(1, xt, ek_vw, kv_vw)
        kv_proj(2, xt, ek_vw, kv_vw)
        kv_proj(3, xt, ek_vw, kv_vw)
        q_proj(1)
        for pr in range(3):
            kv_proj(pr, xt2, ekp_vw, kvp_vw)
        # the last xt2 projection's kvp mul is deferred until after the
        # first stt+recip pair so the den->recip chain does not queue
        # behind it on DVE (kvp3 only gates the last two num-col matmuls)
        kvpsB3 = kv_proj(3, xt2, ekp_vw, kvp_vw, do_mul=False)
        nc.scalar.activation(eq[:, 512:1024], qp[1][:], AF.Exp, scale=-1.0)

        # grid reduction, den parts FIRST (row den, col den, row num,
        # col num) so the combine's den->recip chain overlaps the num
        # matmuls.  Row part writes contiguous out cols per 4-grid-row
        # slice; col part writes strided out cols (token r*32+c).
        GR = dict(start=False, stop=False, skip_group_check=True)
        GK = dict(start=False, stop=False, skip_group_check=True)
        numT_v = numT[:].rearrange("p (r c) -> p c r", c=N)
        denT_v = denT[:].rearrange("p (r c) -> p c r", c=N)
        wcol_gv = wcol_e[:].rearrange("p (g cb r) -> p g cb r", g=8, cb=4)
        for g in range(8):
            gs = slice(g * 128, (g + 1) * 128)
            nc.tensor.matmul(denT[:, gs], ek[:, gs], wrow_e[:, gs], **GR)
        for g in range(8):
            gs = slice(g * 128, (g + 1) * 128)
            nc.tensor.matmul(denT_v[:, 4 * g:4 * (g + 1), :],
                             ekp[:, gs], wcol_gv[:, g], **GK)
        for g in range(8):
            gs = slice(g * 128, (g + 1) * 128)
            nc.tensor.matmul(numT[:, gs], kv[:, gs], wrow_e[:, gs], **GR)
        for g in range(6):
            gs = slice(g * 128, (g + 1) * 128)
            nc.tensor.matmul(numT_v[:, 4 * g:4 * (g + 1), :],
                             kvp[:, gs], wcol_gv[:, g], **GK)

        # combine: out = num * recip(den * (1 + exp(-q))) == sigmoid(q)*num/den
        # stt+recip per half on DVE (den-gated, overlap the num matmuls);
        # the deferred kvp3 mul and the two num-col matmuls it feeds slot
        # in between so the den->recip chain isn't queued behind them.
        nc.vector.scalar_tensor_tensor(
            m1[:, 0:512], eq[:, 0:512], 1.0, denT[:, 0:512], ALU.add, ALU.mult)
        nc.vector.reciprocal_approx_fast(rden[:, 0:512], m1[:, 0:512])
        nc.vector.tensor_mul(kvp_vw[:, 6:8, :], ekp_vw[:, 6:8, :],
                             kvpsB3[:, :, 128:256])
        for g in range(6, 8):
            gs = slice(g * 128, (g + 1) * 128)
            nc.tensor.matmul(numT_v[:, 4 * g:4 * (g + 1), :],
                             kvp[:, gs], wcol_gv[:, g], **GK)
        nc.vector.scalar_tensor_tensor(
            m1[:, 512:1024], eq[:, 512:1024], 1.0, denT[:, 512:1024],
            ALU.add, ALU.mult)
        nc.vector.reciprocal_approx_fast(rden[:, 512:1024], m1[:, 512:1024])
        for qt in range(4):
            qs = slice(qt * 256, (qt + 1) * 256)
            nc.vector.tensor_mul(w2[:, qs], rden[:, qs], numT[:, qs])
            eng = nc.sync if qt % 2 == 0 else nc.scalar
            eng.dma_start(out=out_d[:, qs], in_=w2[:, qs])

    nc.compile()
    return nc


def _get_nc():
    global _CACHED_NC
    if _CACHED_NC is None:
        _CACHED_NC = _build_nc()
    return _CACHED_NC


def _interleave_halves(a):
    """[256, M] -> [128, 2*M] with cols (half, m); partitions = dim%128."""
    return np.concatenate([a[0:128], a[128:256]], axis=1)


def make_shards(x, Wq, Wk, Wv, wbias):
    """Build the per-core input maps (host-side layout/sharding only)."""
    bf = ml_dtypes.bfloat16
    B = x.shape[0]

    # neighbor-weight reorganization: for token t=(r,c), sorted wbias cols are
    #   [0, r)   -> col-neighbor grid-row j = pos
    #   [r, r+N) -> row-neighbor grid-col j = pos - r
    #   [r+N, 2N-1) -> col-neighbor grid-row j = pos - (N - 1)
    Wr = np.empty((T, N), np.float32)
    Wc = np.full((T, N), NEG, np.float32)
    for t in range(T):
        r = t // N
        Wr[t] = wbias[t, r:r + N]
        Wc[t, :r] = wbias[t, :r]
        Wc[t, r + 1:] = wbias[t, r + N:]

    # compact block-diagonal weights (exp-folded), expanded on-device:
    #   wrc[rb*32+j, g*32+c] = exp(Wr[(4g+rb)*32 + c, j])
    #   wcc[rb*32+j, g*32+c] = exp(Wc[c*32 + 4g+rb, j])
    wrc = np.empty((128, 256), np.float32)
    wcc = np.full((128, 256), NEG, np.float32)
    rb, j, c = np.meshgrid(np.arange(4), np.arange(N), np.arange(N),
                           indexing="ij")
    for g in range(8):
        wrc[rb * N + j, g * N + c] = Wr[(4 * g + rb) * N + c, j]
        wcc[rb * N + j, g * N + c] = Wc[c * N + 4 * g + rb, j]
    wrc = np.exp(wrc).astype(bf)
    wcc = np.exp(wcc).astype(bf)

    xt_b, xt2_b = [], []
    for b in range(B):
        xf = x[b].T.astype(bf)                                 # [256, 1024]
        xt_b.append(np.ascontiguousarray(_interleave_halves(xf)))
        # grid-transposed token order t' = c*32 + r
        x2 = np.ascontiguousarray(
            xf.reshape(256, N, N).transpose(0, 2, 1).reshape(256, T))
        xt2_b.append(np.ascontiguousarray(_interleave_halves(x2)))

    in_maps = []
    for core in range(8):
        b, hp = core // 2, core % 2
        sl = slice(hp * 128, (hp + 1) * 128)
        wq_c = _interleave_halves(Wq[sl].T).astype(bf)                # [128,256]
        k_h = Wk[sl].T.reshape(2, 128, 128)
        v_h = Wv[sl].T.reshape(2, 128, 128)
        wkv_c = np.concatenate([k_h[0], v_h[0], k_h[1], v_h[1]],
                               axis=1).astype(bf)                      # [128,512]
        wb1_c = np.ascontiguousarray(
            np.concatenate([wq_c, wkv_c], axis=1))                     # [128,768]
        xh = xt_b[b].reshape(128, 2, 2, 512).transpose(0, 2, 1, 3)
        xh = xh.reshape(128, 2048)
        x2h = xt2_b[b]   # cols already (kh, t'): kh*1024 + t'
        in_maps.append({
            "xth0": np.ascontiguousarray(xh[:, 0:1024]),
            "xth1": np.ascontiguousarray(xh[:, 1024:2048]),
            "xt2h0": np.ascontiguousarray(x2h[:, 0:1024]),
            "xt2h1": np.ascontiguousarray(x2h[:, 1024:2048]),
            "wb1": wb1_c,
            "wrc": wrc,
            "wcc": wcc,
        })
    return in_maps


def kernel(x, Wq, Wk, Wv, wbias, key_indices=None, **_unused):
    global LAST_RESULT
    x = np.asarray(x, np.float32)
    Wq = np.asarray(Wq, np.float32)
    Wk = np.asarray(Wk, np.float32)
    Wv = np.asarray(Wv, np.float32)
    wbias = np.asarray(wbias, np.float32)

    nc = _get_nc()
    in_maps = make_shards(x, Wq, Wk, Wv, wbias)
    try:
        res = run_bass_kernel_spmd(nc, in_maps, core_ids=list(range(8)))
    except ModuleNotFoundError:
        # BASS_TRACE set but the NTFF profile hook module is unavailable in
        # this environment -- rerun untraced
        os.environ["BASS_NEVER_TRACE"] = "1"
        res = run_bass_kernel_spmd(nc, in_maps, core_ids=list(range(8)))
    LAST_RESULT = res

    B = x.shape[0]
    out = np.empty((B, T, D), np.float32)
    for core in range(8):
        b, hp = core // 2, core % 2
        out[b, :, hp * 128:(hp + 1) * 128] = \
            res.results[core]["out"].astype(np.float32).T
    return out
